# revision 9
# baseline (speedup 1.0000x reference)
"""Trainium2 Bass kernel for the 3-block invertible coupling flow (RealNVP-style).

Computation (per sample row of x = [u1(256) | u2(256) | t(1)]):
    for j in 3 blocks:
        v1 = u1 * exp(mlp_s2(u2)) + mlp_t2(u2)
        v2 = u2 * exp(mlp_s1(v1)) + mlp_t1(v1)
        u1, u2 = v1, v2
    out = [u1 | u2 | t]
Each mlp is 256 -> 32 (tanh) -> 256.

v6 strategy (pure data parallel over batch, 131072 -> 8 cores x 16384):
  * fp16 state/weights/IO (PSUM fp32; bf16's 8-bit mantissa fails the 2e-2
    absmax gate through the exp() amplification; fp16's 10-bit passes).
    Host transposes to feature-major [512, bc]; t column stays on host.
  * No hidden duplication: per pair of 512-col batch chains the hidden
    units pack [sA|tA|sB|tB] x 32 across partitions.  L1 is col-tiled
    (M=64 per chain, chains concurrent on col strips 0/64); L2 is
    row-tiled (4x K=32 strips concurrent).
  * ch-major tiles [128, ch, c, BT]: the u*exp multiply slices per chain
    are fully contiguous -> DVE 2x_1P fp16 mode.
  * Per pair-halfstep: ScalarE tanh (FD512) + 2 exp (FD1024, b2s as ACT
    bias); DVE mult chain1 (2x) + 2 stt readouts v=(m+b2t)+t_psum (1x);
    GPSIMD mult chain0.
  * PE warmup burst at program start (overlaps input DMA) so the HAM
    clock-gate opens (matmuls 2.4 GHz, not 1.2).
  * Loop order: (j,q) outer, 16 pairs inner -> all pairs independent.
  * PSUM banks: h bufs2 (2) + s bufs1 (2) + t bufs2 (4) = 8.
"""

from contextlib import ExitStack

import numpy as np

import concourse.bass as bass
import concourse.tile as tile
from concourse import bacc, mybir
from concourse.bass_utils import run_bass_kernel_spmd

F32 = mybir.dt.float32
DT16 = mybir.dt.float16
NP16 = np.float16

B_TOTAL = 131072
D = 512
S = 256
H = 32
L = 3
NCORES = 8
BT = 512          # batch columns per chain (one PSUM bank)
PAIR = 2 * BT     # batch columns per pair

# which chains' u*exp(s) multiply run on GPSIMD (rest on DVE)
GPSIMD_CHAINS = (0,)
WARMUP_MM = 48


def _pack_weights(W1, b1, W2, b2):
    """Host-side repack.

    q=0 updates u1 from u2 (s-idx 1, t-idx 3); q=1 updates u2 from v1
    (s-idx 0, t-idx 2).

    Hidden layout on partitions: [sA(0:32) | tA(32:64) | sB(64:96) | tB(96:128)].
    """
    W1 = np.asarray(W1, np.float32)
    b1 = np.asarray(b1, np.float32)
    W2 = np.asarray(W2, np.float32)
    b2 = np.asarray(b2, np.float32)
    w1p = np.empty((L, 2, 2, 128, 128), np.float32)
    b1p = np.empty((L, 2, 128), np.float32)
    w2p = np.empty((L, 2, 128, 2, 128), np.float32)
    b2sp = np.empty((L, 2, 2, 128), np.float32)   # exp bias, per (fc, feat)
    b2tp = np.empty((L, 2, 2, 128), np.float32)   # t bias, per (fc, feat)
    for j in range(L):
        for q in range(2):
            s_idx, t_idx = (1, 3) if q == 0 else (0, 2)
            for c in range(2):
                blk = slice(c * 128, (c + 1) * 128)
                for g, m_idx in ((0, s_idx), (1, t_idx), (2, s_idx), (3, t_idx)):
                    w1p[j, q, c, :, 32 * g : 32 * (g + 1)] = W1[j, m_idx, blk, :]
            for g, m_idx in ((0, s_idx), (1, t_idx), (2, s_idx), (3, t_idx)):
                b1p[j, q, 32 * g : 32 * (g + 1)] = b1[j, m_idx]
                for fc in range(2):
                    w2p[j, q, 32 * g : 32 * (g + 1), fc, :] = W2[
                        j, m_idx, :, 128 * fc : 128 * (fc + 1)
                    ]
            for fc in range(2):
                b2sp[j, q, fc] = b2[j, s_idx, 128 * fc : 128 * (fc + 1)]
                b2tp[j, q, fc] = b2[j, t_idx, 128 * fc : 128 * (fc + 1)]
    return dict(
        w1p=w1p.astype(NP16),
        b1p=b1p,
        w2p=w2p.astype(NP16),
        b2sp=b2sp,
        b2tp=b2tp,
    )


def build_nc(bc):
    """Per-core Bass program; x_t [512, bc] fp16 feature-major in/out."""
    assert bc % PAIR == 0
    npair = bc // PAIR
    nc = bacc.Bacc(None, target_bir_lowering=False)
    x_d = nc.declare_dram_parameter("x_t", [D, bc], DT16, isOutput=False)
    w1_d = nc.declare_dram_parameter("w1p", [L, 2, 2, 128, 128], DT16, isOutput=False)
    b1_d = nc.declare_dram_parameter("b1p", [L, 2, 128], F32, isOutput=False)
    w2_d = nc.declare_dram_parameter("w2p", [L, 2, 128, 2, 128], DT16, isOutput=False)
    b2s_d = nc.declare_dram_parameter("b2sp", [L, 2, 2, 128], F32, isOutput=False)
    b2t_d = nc.declare_dram_parameter("b2tp", [L, 2, 2, 128], F32, isOutput=False)
    out_d = nc.declare_dram_parameter("out", [D, bc], DT16, isOutput=True)

    TANH = mybir.ActivationFunctionType.Tanh
    EXP = mybir.ActivationFunctionType.Exp
    ADD = mybir.AluOpType.add

    with tile.TileContext(nc) as tc, ExitStack() as ctx:
        singles = ctx.enter_context(tc.tile_pool(name="singles", bufs=1))
        p_state = ctx.enter_context(tc.tile_pool(name="state", bufs=1))
        p_th = ctx.enter_context(tc.tile_pool(name="th", bufs=4))
        p_e = ctx.enter_context(tc.tile_pool(name="e", bufs=3))
        p_m = ctx.enter_context(tc.tile_pool(name="m", bufs=3))
        ps_h = ctx.enter_context(
            tc.tile_pool(name="ps_h", bufs=2, space=bass.MemorySpace.PSUM)
        )
        ps_s = ctx.enter_context(
            tc.tile_pool(name="ps_s", bufs=1, space=bass.MemorySpace.PSUM)
        )
        ps_t = ctx.enter_context(
            tc.tile_pool(name="ps_t", bufs=2, space=bass.MemorySpace.PSUM)
        )

        # --- weights (persist in SBUF) -----------------------------------
        w1s = singles.tile([128, L, 2, 2, 128], DT16)
        nc.gpsimd.dma_start(
            out=w1s[:], in_=w1_d[:].rearrange("j q c p m -> p j q c m")
        )
        b1s = singles.tile([128, L, 2], F32)
        nc.gpsimd.dma_start(out=b1s[:], in_=b1_d[:].rearrange("j q p -> p j q"))
        w2s = singles.tile([128, L, 2, 2, 128], DT16)
        nc.gpsimd.dma_start(
            out=w2s[:], in_=w2_d[:].rearrange("j q p f m -> p j q f m")
        )
        b2ss = singles.tile([128, L, 2, 2], F32)
        nc.gpsimd.dma_start(
            out=b2ss[:], in_=b2s_d[:].rearrange("j q f p -> p j q f")
        )
        b2ts = singles.tile([128, L, 2, 2], F32)
        nc.gpsimd.dma_start(
            out=b2ts[:], in_=b2t_d[:].rearrange("j q f p -> p j q f")
        )

        # --- PE warmup: dense matmul burst while input DMAs stream in ----
        wz = singles.tile([128, BT], DT16)
        nc.vector.memset(wz[:], 0.0)
        for _ in range(WARMUP_MM):
            pw = ps_h.tile([128, BT], F32, tag="h", name="warm")
            nc.tensor.matmul(pw[:], wz[:, 0:128], wz[:])

        # --- state load: u[pr][h] = [128, ch, c, BT] fp16 ----------------
        us = []
        for pr in range(npair):
            b0 = pr * PAIR
            u = []
            for h in range(2):
                ut = p_state.tile(
                    [128, 2, 2, BT], DT16, tag=f"st{h}_{pr}", name=f"u{h}_{pr}"
                )
                for ch in range(2):
                    bch = b0 + ch * BT
                    nc.sync.dma_start(
                        out=ut[:, ch],
                        in_=x_d[h * S : (h + 1) * S, bch : bch + BT].rearrange(
                            "(c p) b -> p c b", p=128
                        ),
                    )
                u.append(ut)
            us.append(u)

        # --- 6 half-steps, software-pipelined with skew ------------------
        # Stage A(pr): L1 -> tanh -> s-matmuls -> exp -> mult
        # Stage B(pr): t-matmuls -> stt readout (writes v)
        # Emission order per halfstep: A(0) A(1) A(2) B(0) A(3) B(1) ...
        # so no queued PE instruction waits long on a PSUM bank drain.
        SKEW = 3
        live = {}

        def stage_a(j, q, pr):
            u = us[pr]
            hin = u[1 - q]   # MLP input state
            tgt = u[q]       # multiplicand state (to be updated)

            # L1: hidden = W1p^T @ hin, chains on col strips 0/64
            ph = ps_h.tile([128, BT], F32, tag="h")
            for ch in range(2):
                for c in range(2):
                    nc.tensor.matmul(
                        ph[64 * ch : 64 * (ch + 1), :],
                        w1s[:, j, q, c, 64 * ch : 64 * (ch + 1)],
                        hin[:, ch, c, :],
                        start=(c == 0),
                        stop=(c == 1),
                        tile_position=(0, 64 * ch),
                    )
            th = p_th.tile([128, BT], DT16, tag="th")
            nc.scalar.activation(th[:], ph[:], TANH, bias=b1s[:, j, q : q + 1])

            # L2 s-matmuls: 4 concurrent row strips; exp per fc
            ee = p_e.tile([128, 2, 2, BT], DT16, tag="ee", name="ee")
            for fc in range(2):
                pss = ps_s.tile([128, 2, BT], F32, tag="s", name=f"pss{fc}")
                for ch in range(2):
                    r = 64 * ch
                    nc.tensor.matmul(
                        pss[:, ch, :],
                        w2s[r : r + 32, j, q, fc, :],
                        th[r : r + 32, :],
                        tile_position=(r, 0),
                    )
                nc.scalar.activation(
                    ee[:, :, fc, :],
                    pss[:],
                    EXP,
                    bias=b2ss[:, j, q, fc : fc + 1],
                )

            # multiplies m = u * ee, split per chain over engines
            m = p_m.tile([128, 2, 2, BT], DT16, tag="m", name="m")
            gp_chains = (0, 1) if pr % 5 == 4 else GPSIMD_CHAINS
            for ch in range(2):
                eng = nc.gpsimd if ch in gp_chains else nc.vector
                eng.tensor_mul(out=m[:, ch], in0=tgt[:, ch], in1=ee[:, ch])
            live[pr] = (th, m)

        def stage_b(j, q, pr):
            th, m = live.pop(pr)
            # L2 t-matmuls + fused readout v = (m + b2t) + t_psum
            v = p_state.tile(
                [128, 2, 2, BT], DT16, tag=f"st{q}_{pr}", name=f"v{pr}"
            )
            for fc in range(2):
                pst = ps_t.tile([128, 2, BT], F32, tag="t", name=f"pst{fc}")
                for ch in range(2):
                    r = 64 * ch + 32
                    nc.tensor.matmul(
                        pst[:, ch, :],
                        w2s[r : r + 32, j, q, fc, :],
                        th[r : r + 32, :],
                        tile_position=(r, 0),
                    )
                nc.vector.scalar_tensor_tensor(
                    out=v[:, :, fc, :],
                    in0=m[:, :, fc, :],
                    scalar=b2ts[:, j, q, fc : fc + 1],
                    in1=pst[:],
                    op0=ADD,
                    op1=ADD,
                )
            us[pr][q] = v

        for j in range(L):
            for q in range(2):
                for pr in range(npair):
                    stage_a(j, q, pr)
                    if pr >= SKEW:
                        stage_b(j, q, pr - SKEW)
                for pr in range(npair - SKEW, npair):
                    stage_b(j, q, pr)

        # --- store -------------------------------------------------------
        for pr in range(npair):
            b0 = pr * PAIR
            for h in range(2):
                for ch in range(2):
                    bch = b0 + ch * BT
                    nc.sync.dma_start(
                        out=out_d[h * S : (h + 1) * S, bch : bch + BT].rearrange(
                            "(c p) b -> p c b", p=128
                        ),
                        in_=us[pr][h][:, ch],
                    )
    nc.compile()
    return nc


_NC_CACHE = {}
TRACE = False
LAST_EXEC_NS = None
LAST_RES = None


def _get_nc(bc):
    if bc not in _NC_CACHE:
        _NC_CACHE[bc] = build_nc(bc)
    return _NC_CACHE[bc]


def kernel(x, W1, b1, W2, b2):
    global LAST_EXEC_NS
    x = np.asarray(x, np.float32)
    b = x.shape[0]
    assert b % NCORES == 0
    bc = b // NCORES
    packed = _pack_weights(W1, b1, W2, b2)
    nc = _get_nc(bc)
    in_maps = [
        {
            "x_t": np.ascontiguousarray(x[i * bc : (i + 1) * bc, :D].T).astype(NP16),
            **packed,
        }
        for i in range(NCORES)
    ]
    res = run_bass_kernel_spmd(nc, in_maps, list(range(NCORES)), trace=TRACE)
    if getattr(res, "exec_time_ns", None):
        LAST_EXEC_NS = res.exec_time_ns
    if TRACE:
        globals()["LAST_RES"] = res
    out = np.empty((b, D + 1), np.float32)
    for i in range(NCORES):
        out[i * bc : (i + 1) * bc, :D] = res.results[i]["out"].astype(np.float32).T
    out[:, D] = x[:, D]
    return out


# revision 13
# speedup vs baseline: 1.1075x; 1.1075x over previous
"""Trainium2 Bass kernel for the 3-block invertible coupling flow (RealNVP-style).

Computation (per sample row of x = [u1(256) | u2(256) | t(1)]):
    for j in 3 blocks:
        v1 = u1 * exp(mlp_s2(u2)) + mlp_t2(u2)
        v2 = u2 * exp(mlp_s1(v1)) + mlp_t1(v1)
        u1, u2 = v1, v2
    out = [u1 | u2 | t]
Each mlp is 256 -> 32 (tanh) -> 256.

v6 strategy (pure data parallel over batch, 131072 -> 8 cores x 16384):
  * fp16 state/weights/IO (PSUM fp32; bf16's 8-bit mantissa fails the 2e-2
    absmax gate through the exp() amplification; fp16's 10-bit passes).
    Host transposes to feature-major [512, bc]; t column stays on host.
  * No hidden duplication: per pair of 512-col batch chains the hidden
    units pack [sA|tA|sB|tB] x 32 across partitions.  L1 is col-tiled
    (M=64 per chain, chains concurrent on col strips 0/64); L2 is
    row-tiled (4x K=32 strips concurrent).
  * ch-major tiles [128, ch, c, BT]: the u*exp multiply slices per chain
    are fully contiguous -> DVE 2x_1P fp16 mode.
  * Per pair-halfstep: ScalarE tanh (FD512) + 2 exp (FD1024, b2s as ACT
    bias); DVE mult chain1 (2x) + 2 stt readouts v=(m+b2t)+t_psum (1x);
    GPSIMD mult chain0.
  * PE warmup burst at program start (overlaps input DMA) so the HAM
    clock-gate opens (matmuls 2.4 GHz, not 1.2).
  * Loop order: (j,q) outer, 16 pairs inner -> all pairs independent.
  * PSUM banks: h bufs2 (2) + s bufs1 (2) + t bufs2 (4) = 8.
"""

from contextlib import ExitStack

import numpy as np

import concourse.bass as bass
import concourse.tile as tile
from concourse import bacc, mybir
from concourse.bass_utils import run_bass_kernel_spmd

F32 = mybir.dt.float32
DT16 = mybir.dt.float16
NP16 = np.float16

B_TOTAL = 131072
D = 512
S = 256
H = 32
L = 3
NCORES = 8
BT = 512          # batch columns per chain (one PSUM bank)
PAIR = 2 * BT     # batch columns per pair

# which chains' u*exp(s) multiply run on GPSIMD (rest on DVE)
GPSIMD_CHAINS = (0,)
WARMUP_MM = 24


def _pack_weights(W1, b1, W2, b2):
    """Host-side repack.

    q=0 updates u1 from u2 (s-idx 1, t-idx 3); q=1 updates u2 from v1
    (s-idx 0, t-idx 2).

    Hidden layout on partitions: [sA(0:32) | tA(32:64) | sB(64:96) | tB(96:128)].
    """
    W1 = np.asarray(W1, np.float32)
    b1 = np.asarray(b1, np.float32)
    W2 = np.asarray(W2, np.float32)
    b2 = np.asarray(b2, np.float32)
    w1p = np.empty((L, 2, 2, 128, 128), np.float32)
    b1p = np.empty((L, 2, 128), np.float32)
    w2p = np.empty((L, 2, 128, 2, 128), np.float32)
    b2sp = np.empty((L, 2, 2, 128), np.float32)   # exp bias, per (fc, feat)
    b2tp = np.empty((L, 2, 2, 128), np.float32)   # t bias, per (fc, feat)
    for j in range(L):
        for q in range(2):
            s_idx, t_idx = (1, 3) if q == 0 else (0, 2)
            for c in range(2):
                blk = slice(c * 128, (c + 1) * 128)
                for g, m_idx in ((0, s_idx), (1, t_idx), (2, s_idx), (3, t_idx)):
                    w1p[j, q, c, :, 32 * g : 32 * (g + 1)] = W1[j, m_idx, blk, :]
            for g, m_idx in ((0, s_idx), (1, t_idx), (2, s_idx), (3, t_idx)):
                b1p[j, q, 32 * g : 32 * (g + 1)] = b1[j, m_idx]
                for fc in range(2):
                    w2p[j, q, 32 * g : 32 * (g + 1), fc, :] = W2[
                        j, m_idx, :, 128 * fc : 128 * (fc + 1)
                    ]
            for fc in range(2):
                b2sp[j, q, fc] = b2[j, s_idx, 128 * fc : 128 * (fc + 1)]
                b2tp[j, q, fc] = b2[j, t_idx, 128 * fc : 128 * (fc + 1)]
    return dict(
        w1p=w1p.astype(NP16),
        b1p=b1p,
        w2p=w2p.astype(NP16),
        b2sp=b2sp,
        b2tp=b2tp,
    )


def build_nc(bc):
    """Per-core Bass program; x_t [512, bc] fp16 feature-major in/out."""
    assert bc % PAIR == 0
    npair = bc // PAIR
    nc = bacc.Bacc(None, target_bir_lowering=False)
    x_d = nc.declare_dram_parameter("x_t", [D, bc], DT16, isOutput=False)
    w1_d = nc.declare_dram_parameter("w1p", [L, 2, 2, 128, 128], DT16, isOutput=False)
    b1_d = nc.declare_dram_parameter("b1p", [L, 2, 128], F32, isOutput=False)
    w2_d = nc.declare_dram_parameter("w2p", [L, 2, 128, 2, 128], DT16, isOutput=False)
    b2s_d = nc.declare_dram_parameter("b2sp", [L, 2, 2, 128], F32, isOutput=False)
    b2t_d = nc.declare_dram_parameter("b2tp", [L, 2, 2, 128], F32, isOutput=False)
    out_d = nc.declare_dram_parameter("out", [D, bc], DT16, isOutput=True)

    TANH = mybir.ActivationFunctionType.Tanh
    EXP = mybir.ActivationFunctionType.Exp
    ADD = mybir.AluOpType.add

    with tile.TileContext(nc) as tc, ExitStack() as ctx:
        singles = ctx.enter_context(tc.tile_pool(name="singles", bufs=1))
        p_state = ctx.enter_context(tc.tile_pool(name="state", bufs=1))
        p_th = ctx.enter_context(tc.tile_pool(name="th", bufs=6))
        p_e = ctx.enter_context(tc.tile_pool(name="e", bufs=5))
        p_m = ctx.enter_context(tc.tile_pool(name="m", bufs=5))
        ps_h = ctx.enter_context(
            tc.tile_pool(name="ps_h", bufs=2, space=bass.MemorySpace.PSUM)
        )
        ps_s = ctx.enter_context(
            tc.tile_pool(name="ps_s", bufs=2, space=bass.MemorySpace.PSUM)
        )
        ps_t = ctx.enter_context(
            tc.tile_pool(name="ps_t", bufs=1, space=bass.MemorySpace.PSUM)
        )

        # --- weights (persist in SBUF) -----------------------------------
        w1s = singles.tile([128, L, 2, 2, 128], DT16)
        nc.gpsimd.dma_start(
            out=w1s[:], in_=w1_d[:].rearrange("j q c p m -> p j q c m")
        )
        b1s = singles.tile([128, L, 2], F32)
        nc.gpsimd.dma_start(out=b1s[:], in_=b1_d[:].rearrange("j q p -> p j q"))
        w2s = singles.tile([128, L, 2, 2, 128], DT16)
        nc.gpsimd.dma_start(
            out=w2s[:], in_=w2_d[:].rearrange("j q p f m -> p j q f m")
        )
        b2ss = singles.tile([128, L, 2, 2], F32)
        nc.gpsimd.dma_start(
            out=b2ss[:], in_=b2s_d[:].rearrange("j q f p -> p j q f")
        )
        b2ts = singles.tile([128, L, 2, 2], F32)
        nc.gpsimd.dma_start(
            out=b2ts[:], in_=b2t_d[:].rearrange("j q f p -> p j q f")
        )

        # --- PE warmup: dense matmul burst while input DMAs stream in ----
        wz = singles.tile([128, BT], DT16)
        nc.vector.memset(wz[:], 0.0)
        for _ in range(WARMUP_MM):
            pw = ps_h.tile([128, BT], F32, tag="h", name="warm")
            nc.tensor.matmul(pw[:], wz[:, 0:128], wz[:])

        # --- state load: u[pr][h] = [128, ch, c, BT] fp16 ----------------
        us = []
        for pr in range(npair):
            b0 = pr * PAIR
            u = []
            for h in range(2):
                ut = p_state.tile(
                    [128, 2, 2, BT], DT16, tag=f"st{h}_{pr}", name=f"u{h}_{pr}"
                )
                for ch in range(2):
                    bch = b0 + ch * BT
                    nc.sync.dma_start(
                        out=ut[:, ch],
                        in_=x_d[h * S : (h + 1) * S, bch : bch + BT].rearrange(
                            "(c p) b -> p c b", p=128
                        ),
                    )
                u.append(ut)
            us.append(u)

        # --- 6 half-steps, software-pipelined with skew ------------------
        # Stage A(pr): L1 -> tanh -> s-matmuls -> exp -> mult
        # Stage B(pr): t-matmuls -> stt readout (writes v)
        # Emission order per halfstep: A(0) A(1) A(2) B(0) A(3) B(1) ...
        # so no queued PE instruction waits long on a PSUM bank drain.
        SKEW = 2
        live = {}

        def stage_a(j, q, pr):
            u = us[pr]
            hin = u[1 - q]   # MLP input state
            tgt = u[q]       # multiplicand state (to be updated)

            # L1: hidden = W1p^T @ hin, chains on col strips 0/64
            ph = ps_h.tile([128, BT], F32, tag="h")
            for ch in range(2):
                for c in range(2):
                    nc.tensor.matmul(
                        ph[64 * ch : 64 * (ch + 1), :],
                        w1s[:, j, q, c, 64 * ch : 64 * (ch + 1)],
                        hin[:, ch, c, :],
                        start=(c == 0),
                        stop=(c == 1),
                        tile_position=(0, 64 * ch),
                    )
            th = p_th.tile([128, BT], DT16, tag="th")
            nc.scalar.activation(th[:], ph[:], TANH, bias=b1s[:, j, q : q + 1])

            # L2 s-matmuls: 4 concurrent row strips; exp per fc
            ee = p_e.tile([128, 2, 2, BT], DT16, tag="ee", name="ee")
            for fc in range(2):
                pss = ps_s.tile([128, 2, BT], F32, tag="s", name=f"pss{fc}")
                for ch in range(2):
                    r = 64 * ch
                    nc.tensor.matmul(
                        pss[:, ch, :],
                        w2s[r : r + 32, j, q, fc, :],
                        th[r : r + 32, :],
                        tile_position=(r, 0),
                    )
                nc.scalar.activation(
                    ee[:, :, fc, :],
                    pss[:],
                    EXP,
                    bias=b2ss[:, j, q, fc : fc + 1],
                )

            # multiplies m = u * ee, split per chain over engines
            m = p_m.tile([128, 2, 2, BT], DT16, tag="m", name="m")
            for ch in range(2):
                eng = nc.gpsimd if ch in GPSIMD_CHAINS else nc.vector
                eng.tensor_mul(out=m[:, ch], in0=tgt[:, ch], in1=ee[:, ch])
            live[pr] = (th, m)

        def stage_b(j, q, pr):
            th, m = live.pop(pr)
            # L2 t-matmuls + fused readout v = (m + b2t) + t_psum
            v = p_state.tile(
                [128, 2, 2, BT], DT16, tag=f"st{q}_{pr}", name=f"v{pr}"
            )
            for fc in range(2):
                pst = ps_t.tile([128, 2, BT], F32, tag="t", name=f"pst{fc}")
                for ch in range(2):
                    r = 64 * ch + 32
                    nc.tensor.matmul(
                        pst[:, ch, :],
                        w2s[r : r + 32, j, q, fc, :],
                        th[r : r + 32, :],
                        tile_position=(r, 0),
                    )
                nc.vector.scalar_tensor_tensor(
                    out=v[:, :, fc, :],
                    in0=m[:, :, fc, :],
                    scalar=b2ts[:, j, q, fc : fc + 1],
                    in1=pst[:],
                    op0=ADD,
                    op1=ADD,
                )
            us[pr][q] = v

        for j in range(L):
            for q in range(2):
                for pr in range(npair):
                    stage_a(j, q, pr)
                    # keep the PE fed through pipeline fill so the HAM
                    # clock-gate stays open (one >3.4us idle would re-throttle
                    # to 1.2 GHz for the rest of the kernel)
                    if j == 0 and q == 0 and pr < 10:
                        for _ in range(3):
                            pw = ps_h.tile([128, BT], F32, tag="h", name="wf")
                            nc.tensor.matmul(pw[:], wz[:, 0:128], wz[:])
                    if pr >= SKEW:
                        stage_b(j, q, pr - SKEW)
                for pr in range(npair - SKEW, npair):
                    stage_b(j, q, pr)

        # --- store -------------------------------------------------------
        for pr in range(npair):
            b0 = pr * PAIR
            for h in range(2):
                for ch in range(2):
                    bch = b0 + ch * BT
                    nc.sync.dma_start(
                        out=out_d[h * S : (h + 1) * S, bch : bch + BT].rearrange(
                            "(c p) b -> p c b", p=128
                        ),
                        in_=us[pr][h][:, ch],
                    )
    nc.compile()
    return nc


_NC_CACHE = {}
TRACE = False
LAST_EXEC_NS = None
LAST_RES = None


def _get_nc(bc):
    if bc not in _NC_CACHE:
        _NC_CACHE[bc] = build_nc(bc)
    return _NC_CACHE[bc]


def kernel(x, W1, b1, W2, b2):
    global LAST_EXEC_NS
    x = np.asarray(x, np.float32)
    b = x.shape[0]
    assert b % NCORES == 0
    bc = b // NCORES
    packed = _pack_weights(W1, b1, W2, b2)
    nc = _get_nc(bc)
    in_maps = [
        {
            "x_t": np.ascontiguousarray(x[i * bc : (i + 1) * bc, :D].T).astype(NP16),
            **packed,
        }
        for i in range(NCORES)
    ]
    res = run_bass_kernel_spmd(nc, in_maps, list(range(NCORES)), trace=TRACE)
    if getattr(res, "exec_time_ns", None):
        LAST_EXEC_NS = res.exec_time_ns
    if TRACE:
        globals()["LAST_RES"] = res
    out = np.empty((b, D + 1), np.float32)
    for i in range(NCORES):
        out[i * bc : (i + 1) * bc, :D] = res.results[i]["out"].astype(np.float32).T
    out[:, D] = x[:, D]
    return out


# revision 14
# speedup vs baseline: 1.1164x; 1.0080x over previous
"""Trainium2 Bass kernel for the 3-block invertible coupling flow (RealNVP-style).

Computation (per sample row of x = [u1(256) | u2(256) | t(1)]):
    for j in 3 blocks:
        v1 = u1 * exp(mlp_s2(u2)) + mlp_t2(u2)
        v2 = u2 * exp(mlp_s1(v1)) + mlp_t1(v1)
        u1, u2 = v1, v2
    out = [u1 | u2 | t]
Each mlp is 256 -> 32 (tanh) -> 256.

v10 strategy (pure data parallel over batch, 131072 -> 8 cores x 16384):
  * fp16 state/weights/IO (PSUM fp32; bf16's 8-bit mantissa fails the 2e-2
    absmax gate through the exp() amplification; fp16's 10-bit passes).
    Host transposes to feature-major [512, bc]; t column stays on host.
  * No hidden duplication: per pair of 512-col batch chains the hidden
    units pack [sA|tA|sB|tB] x 32 across partitions.  L1 is col-tiled
    (M=64 per chain, chains concurrent on col strips 0/64); L2 is
    row-tiled (4x K=32 strips concurrent).
  * ch-major tiles [128, ch, c, BT]: the u*exp multiply slices per chain
    are fully contiguous -> DVE 2x_1P fp16 mode.
  * Per pair-halfstep: ScalarE tanh (FD512) + 2 exp (FD1024, b2s as ACT
    bias); DVE mult chain1 (2x) + 2 stt readouts v=(m+b2t)+t_psum (1x);
    GPSIMD mult chain0.
  * Loop order: (j,q) outer, 16 pairs inner -> all pairs independent;
    t-matmuls/stt readouts software-pipelined 2 pairs behind (SKEW) so no
    queued PE instruction waits long on a PSUM-bank drain.
  * PE warmup burst + fill-phase filler matmuls keep the HAM clock-gate
    open as long as possible (steady state runs ~72% PE-busy, which this
    silicon's HAM still throttles to 1.2 GHz; the dominant engines are
    DVE/ScalarE regardless).
  * PSUM banks: h bufs2 (2) + s bufs2 (4) + t bufs1 (2) = 8.
"""

from contextlib import ExitStack

import numpy as np

import concourse.bass as bass
import concourse.tile as tile
from concourse import bacc, mybir
from concourse.bass_utils import run_bass_kernel_spmd

F32 = mybir.dt.float32
DT16 = mybir.dt.float16
NP16 = np.float16

B_TOTAL = 131072
D = 512
S = 256
H = 32
L = 3
NCORES = 8
BT = 512          # batch columns per chain (one PSUM bank)
PAIR = 2 * BT     # batch columns per pair

# which chains' u*exp(s) multiply run on GPSIMD (rest on DVE)
GPSIMD_CHAINS = (0,)
WARMUP_MM = 24


def _pack_weights(W1, b1, W2, b2):
    """Host-side repack.

    q=0 updates u1 from u2 (s-idx 1, t-idx 3); q=1 updates u2 from v1
    (s-idx 0, t-idx 2).

    Hidden layout on partitions: [sA(0:32) | tA(32:64) | sB(64:96) | tB(96:128)].
    """
    W1 = np.asarray(W1, np.float32)
    b1 = np.asarray(b1, np.float32)
    W2 = np.asarray(W2, np.float32)
    b2 = np.asarray(b2, np.float32)
    w1p = np.empty((L, 2, 2, 128, 128), np.float32)
    b1p = np.empty((L, 2, 128), np.float32)
    w2p = np.empty((L, 2, 128, 2, 128), np.float32)
    b2sp = np.empty((L, 2, 2, 128), np.float32)   # exp bias, per (fc, feat)
    b2tp = np.empty((L, 2, 2, 128), np.float32)   # t bias, per (fc, feat)
    for j in range(L):
        for q in range(2):
            s_idx, t_idx = (1, 3) if q == 0 else (0, 2)
            for c in range(2):
                blk = slice(c * 128, (c + 1) * 128)
                for g, m_idx in ((0, s_idx), (1, t_idx), (2, s_idx), (3, t_idx)):
                    w1p[j, q, c, :, 32 * g : 32 * (g + 1)] = W1[j, m_idx, blk, :]
            for g, m_idx in ((0, s_idx), (1, t_idx), (2, s_idx), (3, t_idx)):
                b1p[j, q, 32 * g : 32 * (g + 1)] = b1[j, m_idx]
                for fc in range(2):
                    w2p[j, q, 32 * g : 32 * (g + 1), fc, :] = W2[
                        j, m_idx, :, 128 * fc : 128 * (fc + 1)
                    ]
            for fc in range(2):
                b2sp[j, q, fc] = b2[j, s_idx, 128 * fc : 128 * (fc + 1)]
                b2tp[j, q, fc] = b2[j, t_idx, 128 * fc : 128 * (fc + 1)]
    return dict(
        w1p=w1p.astype(NP16),
        b1p=b1p,
        w2p=w2p.astype(NP16),
        b2sp=b2sp,
        b2tp=b2tp,
    )


def build_nc(bc):
    """Per-core Bass program; x_t [512, bc] fp16 feature-major in/out."""
    assert bc % PAIR == 0
    npair = bc // PAIR
    nc = bacc.Bacc(None, target_bir_lowering=False)
    x_d = nc.declare_dram_parameter("x_t", [D, bc], DT16, isOutput=False)
    w1_d = nc.declare_dram_parameter("w1p", [L, 2, 2, 128, 128], DT16, isOutput=False)
    b1_d = nc.declare_dram_parameter("b1p", [L, 2, 128], F32, isOutput=False)
    w2_d = nc.declare_dram_parameter("w2p", [L, 2, 128, 2, 128], DT16, isOutput=False)
    b2s_d = nc.declare_dram_parameter("b2sp", [L, 2, 2, 128], F32, isOutput=False)
    b2t_d = nc.declare_dram_parameter("b2tp", [L, 2, 2, 128], F32, isOutput=False)
    out_d = nc.declare_dram_parameter("out", [D, bc], DT16, isOutput=True)

    TANH = mybir.ActivationFunctionType.Tanh
    EXP = mybir.ActivationFunctionType.Exp
    ADD = mybir.AluOpType.add

    with tile.TileContext(nc) as tc, ExitStack() as ctx:
        singles = ctx.enter_context(tc.tile_pool(name="singles", bufs=1))
        p_state = ctx.enter_context(tc.tile_pool(name="state", bufs=1))
        p_th = ctx.enter_context(tc.tile_pool(name="th", bufs=4))
        p_e = ctx.enter_context(tc.tile_pool(name="e", bufs=3))
        p_m = ctx.enter_context(tc.tile_pool(name="m", bufs=3))
        ps_h = ctx.enter_context(
            tc.tile_pool(name="ps_h", bufs=2, space=bass.MemorySpace.PSUM)
        )
        ps_s = ctx.enter_context(
            tc.tile_pool(name="ps_s", bufs=2, space=bass.MemorySpace.PSUM)
        )
        ps_t = ctx.enter_context(
            tc.tile_pool(name="ps_t", bufs=1, space=bass.MemorySpace.PSUM)
        )

        # --- weights (persist in SBUF) -----------------------------------
        w1s = singles.tile([128, L, 2, 2, 128], DT16)
        nc.gpsimd.dma_start(
            out=w1s[:], in_=w1_d[:].rearrange("j q c p m -> p j q c m")
        )
        b1s = singles.tile([128, L, 2], F32)
        nc.gpsimd.dma_start(out=b1s[:], in_=b1_d[:].rearrange("j q p -> p j q"))
        w2s = singles.tile([128, L, 2, 2, 128], DT16)
        nc.gpsimd.dma_start(
            out=w2s[:], in_=w2_d[:].rearrange("j q p f m -> p j q f m")
        )
        b2ss = singles.tile([128, L, 2, 2], F32)
        nc.gpsimd.dma_start(
            out=b2ss[:], in_=b2s_d[:].rearrange("j q f p -> p j q f")
        )
        b2ts = singles.tile([128, L, 2, 2], F32)
        nc.gpsimd.dma_start(
            out=b2ts[:], in_=b2t_d[:].rearrange("j q f p -> p j q f")
        )

        # --- PE warmup: dense matmul burst while input DMAs stream in ----
        wz = singles.tile([128, BT], DT16)
        nc.vector.memset(wz[:], 0.0)
        for _ in range(WARMUP_MM):
            pw = ps_h.tile([128, BT], F32, tag="h", name="warm")
            nc.tensor.matmul(pw[:], wz[:, 0:128], wz[:])

        # --- state load: u[pr][h] = [128, ch, c, BT] fp16 ----------------
        us = []
        for pr in range(npair):
            b0 = pr * PAIR
            u = []
            for h in range(2):
                ut = p_state.tile(
                    [128, 2, 2, BT], DT16, tag=f"st{h}_{pr}", name=f"u{h}_{pr}"
                )
                for ch in range(2):
                    bch = b0 + ch * BT
                    nc.sync.dma_start(
                        out=ut[:, ch],
                        in_=x_d[h * S : (h + 1) * S, bch : bch + BT].rearrange(
                            "(c p) b -> p c b", p=128
                        ),
                    )
                u.append(ut)
            us.append(u)

        # --- 6 half-steps, software-pipelined with skew ------------------
        # Stage A(pr): L1 -> tanh -> s-matmuls -> exp -> mult
        # Stage B(pr): t-matmuls -> stt readout (writes v)
        # Emission order per halfstep: A(0) A(1) A(2) B(0) A(3) B(1) ...
        # so no queued PE instruction waits long on a PSUM bank drain.
        SKEW = 2
        live = {}

        def stage_a(j, q, pr):
            u = us[pr]
            hin = u[1 - q]   # MLP input state
            tgt = u[q]       # multiplicand state (to be updated)

            # L1: hidden = W1p^T @ hin, chains on col strips 0/64
            ph = ps_h.tile([128, BT], F32, tag="h")
            for ch in range(2):
                for c in range(2):
                    nc.tensor.matmul(
                        ph[64 * ch : 64 * (ch + 1), :],
                        w1s[:, j, q, c, 64 * ch : 64 * (ch + 1)],
                        hin[:, ch, c, :],
                        start=(c == 0),
                        stop=(c == 1),
                        tile_position=(0, 64 * ch),
                    )
            th = p_th.tile([128, BT], DT16, tag="th")
            nc.scalar.activation(th[:], ph[:], TANH, bias=b1s[:, j, q : q + 1])

            # L2 s-matmuls: 4 concurrent row strips; exp per fc
            ee = p_e.tile([128, 2, 2, BT], DT16, tag="ee", name="ee")
            for fc in range(2):
                pss = ps_s.tile([128, 2, BT], F32, tag="s", name=f"pss{fc}")
                for ch in range(2):
                    r = 64 * ch
                    nc.tensor.matmul(
                        pss[:, ch, :],
                        w2s[r : r + 32, j, q, fc, :],
                        th[r : r + 32, :],
                        tile_position=(r, 0),
                    )
                nc.scalar.activation(
                    ee[:, :, fc, :],
                    pss[:],
                    EXP,
                    bias=b2ss[:, j, q, fc : fc + 1],
                )

            # multiplies m = u * ee, split per chain over engines
            m = p_m.tile([128, 2, 2, BT], DT16, tag="m", name="m")
            for ch in range(2):
                eng = nc.gpsimd if ch in GPSIMD_CHAINS else nc.vector
                eng.tensor_mul(out=m[:, ch], in0=tgt[:, ch], in1=ee[:, ch])
            live[pr] = (th, m)

        def stage_b(j, q, pr):
            th, m = live.pop(pr)
            # L2 t-matmuls + fused readout v = (m + b2t) + t_psum
            v = p_state.tile(
                [128, 2, 2, BT], DT16, tag=f"st{q}_{pr}", name=f"v{pr}"
            )
            for fc in range(2):
                pst = ps_t.tile([128, 2, BT], F32, tag="t", name=f"pst{fc}")
                for ch in range(2):
                    r = 64 * ch + 32
                    nc.tensor.matmul(
                        pst[:, ch, :],
                        w2s[r : r + 32, j, q, fc, :],
                        th[r : r + 32, :],
                        tile_position=(r, 0),
                    )
                nc.vector.scalar_tensor_tensor(
                    out=v[:, :, fc, :],
                    in0=m[:, :, fc, :],
                    scalar=b2ts[:, j, q, fc : fc + 1],
                    in1=pst[:],
                    op0=ADD,
                    op1=ADD,
                )
            us[pr][q] = v

        for j in range(L):
            for q in range(2):
                for pr in range(npair):
                    stage_a(j, q, pr)
                    # keep the PE fed through pipeline fill so the HAM
                    # clock-gate stays open (one >3.4us idle would re-throttle
                    # to 1.2 GHz for the rest of the kernel)
                    if j == 0 and q == 0 and pr < 10:
                        for _ in range(3):
                            pw = ps_h.tile([128, BT], F32, tag="h", name="wf")
                            nc.tensor.matmul(pw[:], wz[:, 0:128], wz[:])
                    if pr >= SKEW:
                        stage_b(j, q, pr - SKEW)
                for pr in range(npair - SKEW, npair):
                    stage_b(j, q, pr)

        # --- store -------------------------------------------------------
        for pr in range(npair):
            b0 = pr * PAIR
            for h in range(2):
                for ch in range(2):
                    bch = b0 + ch * BT
                    nc.sync.dma_start(
                        out=out_d[h * S : (h + 1) * S, bch : bch + BT].rearrange(
                            "(c p) b -> p c b", p=128
                        ),
                        in_=us[pr][h][:, ch],
                    )
    nc.compile()
    return nc


_NC_CACHE = {}
TRACE = False
LAST_EXEC_NS = None
LAST_RES = None


def _get_nc(bc):
    if bc not in _NC_CACHE:
        _NC_CACHE[bc] = build_nc(bc)
    return _NC_CACHE[bc]


def kernel(x, W1, b1, W2, b2):
    global LAST_EXEC_NS
    x = np.asarray(x, np.float32)
    b = x.shape[0]
    assert b % NCORES == 0
    bc = b // NCORES
    packed = _pack_weights(W1, b1, W2, b2)
    nc = _get_nc(bc)
    in_maps = [
        {
            "x_t": np.ascontiguousarray(x[i * bc : (i + 1) * bc, :D].T).astype(NP16),
            **packed,
        }
        for i in range(NCORES)
    ]
    res = run_bass_kernel_spmd(nc, in_maps, list(range(NCORES)), trace=TRACE)
    if getattr(res, "exec_time_ns", None):
        LAST_EXEC_NS = res.exec_time_ns
    if TRACE:
        globals()["LAST_RES"] = res
    out = np.empty((b, D + 1), np.float32)
    for i in range(NCORES):
        out[i * bc : (i + 1) * bc, :D] = res.results[i]["out"].astype(np.float32).T
    out[:, D] = x[:, D]
    return out
